# Initial kernel scaffold
#
"""Bass/Tile kernel for nn_MultiHeadAttention (B=2, S=2048, D=1024, H=16) on 8 trn2 cores.

Sharding: core c -> (b = c//4, head-group hg = c%4). Each core computes 4 heads'
q/k/v projections, relu-attention, and a partial FC (256 of 1024 contraction rows).
Host pre-casts to bf16, pre-arranges x / weight slices, and sums the 4
partials per batch + bias.

v6 design notes:
  - scores: the two heads of a head-pair run CONCURRENTLY in disjoint 64-row
    PE quadrants (tile_position from base partitions), writing one 2-bank
    PSUM tile; ONE 1024-wide relu drains the pair. Vector gets 7 and Scalar 9
    of the 16 relus per m-loop (scalar is faster per element).
  - av: two heads run concurrently in disjoint 64-col output quadrants.
  - attention m-loops are relu/PE balanced (~9.4us each); all remaining PE
    work (v-projections, fc, next q-projection) is injected into the loops.
  - fc computed as (sc)-pairs: 4 matmuls -> 2-bank PSUM -> one 1024-wide
    gpsimd cast-copy -> one 256KB bf16 store with 2KB lines.
  - x arrives as 4 contiguous 1MB nb-block DMAs (8KB/partition lines) on the
    sync hardware queue behind wk; wq+wv ride the scalar hardware queue.
  - y is bf16; host sums partials in fp64.
"""
import numpy as np
import ml_dtypes

import concourse.bass as bass
import concourse.mybir as mybir
import concourse.tile as tile

F32 = mybir.dt.float32
BF16 = mybir.dt.bfloat16
ts, ds = bass.ts, bass.ds

S = 2048
D = 1024
DL = 256      # per-core q/k/v dim (4 heads x 64)
P = 128
KD = D // P   # 8 k-chunks for projections
SQ = 512      # q-block (matmul N)
NQB = S // SQ # 4
NM = S // P   # 16 kpos chunks
DLC = DL // P # 2


def split_excess_waits(nc, max_embed: int = 1):
    """walrus core_v3 codegen accepts at most one sync-wait per instruction;
    move extra waits onto standalone event-sem instructions inserted before."""
    n_split = 0
    counter = 0
    for f in nc.m.functions:
        for blk in f.blocks:
            insts = blk.instructions
            if not any(
                ins.sync_info is not None and len(ins.sync_info.on_wait) > max_embed
                for ins in insts
            ):
                continue
            newl = []
            for ins in insts:
                si = ins.sync_info
                if si is not None and len(si.on_wait) > max_embed:
                    waits = list(si.on_wait)
                    extra, keep = waits[:-max_embed], waits[-max_embed:]
                    for w in extra:
                        counter += 1
                        es = mybir.InstEventSemaphore(name=f"waitsplit_{counter}")
                        es.engine = ins.engine
                        es.sync_info = mybir.SyncInfo(on_wait=[w], on_update=[])
                        newl.append(es)
                        n_split += 1
                    si.on_wait = keep
                newl.append(ins)
            blk.instructions = newl
    return n_split


def build_nc(with_mask: bool):
    nc = bass.Bass()
    # pre-arranged on host: x[p, nb, k, sq] = x.T[128k+p, 512nb+sq] (one
    # contiguous 1MB block per q-block); w[p, c, f] = w.T[128c+p, f]
    xT = nc.dram_tensor("xT", [P, NQB, KD, SQ], BF16, kind="ExternalInput")
    wq = nc.dram_tensor("wq", [P, DLC, KD, P], BF16, kind="ExternalInput")
    wk = nc.dram_tensor("wk", [P, DLC, KD, P], BF16, kind="ExternalInput")
    wv = nc.dram_tensor("wv", [P, KD, DL], BF16, kind="ExternalInput")
    wfc = nc.dram_tensor("wfc", [P, DLC, D], BF16, kind="ExternalInput")
    maskT = nc.dram_tensor("maskT", [S, S], F32, kind="ExternalInput") if with_mask else None
    # bf16 output halves the store traffic; partials are summed in fp64 on host
    y = nc.dram_tensor("y", [S, D], BF16, kind="ExternalOutput")

    with tile.TileContext(nc) as tc:
        _Emitter(tc, xT, wq, wk, wv, wfc, maskT, y).run()
    split_excess_waits(nc)
    return nc


class _Emitter:
    def __init__(self, tc, xT, wq, wk, wv, wfc, maskT, y):
        self.tc = tc
        self.nc = tc.nc
        self.xT, self.wq, self.wk, self.wv, self.wfc = xT, wq, wk, wv, wfc
        self.maskT, self.y = maskT, y
        self.cp = 0

    # -- engine helpers -----------------------------------------------------
    def copyback(self, out_ap, in_ap):
        if self.cp % 2 == 0:
            self.nc.vector.tensor_copy(out_ap, in_ap)
        else:
            self.nc.scalar.copy(out_ap, in_ap)
        self.cp += 1

    # -- emission pieces ----------------------------------------------------
    def kq_group(self, wsb, dstT, c, nb):
        """one projection psum group: dstT[:, c, nb*SQ:...] via 8 k-chunk matmuls"""
        nc = self.nc
        pt = self.ps_fc.tile([P, SQ], F32, tag="fc", name=f"pj_{dstT.name}_{c}_{nb}")
        for k in range(KD):
            nc.tensor.matmul(
                pt[:], wsb[:, c, k, :], self.xb[:, nb, k, :],
                start=(k == 0), stop=(k == KD - 1),
            )
        self.copyback(dstT[:, c, ds(nb * SQ, SQ)], pt[:])

    def v_group(self, sc):
        nc = self.nc
        pt = self.ps_fc.tile([P, DL], F32, tag="fc", name=f"v_{sc}")
        for k in range(KD):
            nc.tensor.matmul(
                pt[:], self.xb[:, sc // 4, k, ds((sc % 4) * P, P)], self.wv_sb[:, k, :],
                start=(k == 0), stop=(k == KD - 1),
            )
        self.copyback(self.vN[:, sc, :], pt[:])

    def scores_pair(self, qb, hp, m, attn_t, mtile):
        """both heads' score matmuls run concurrently in disjoint PE
        row-quadrants; separate 1-bank tiles + 512-wide relus keep the
        drain latency low enough for the 4-slot psum rotation"""
        nc = self.nc
        pts = []
        for h in range(2):
            pt = self.ps_sc.tile([P, SQ], F32, tag="sc", name=f"sc_{qb}_{hp}_{m}_{h}")
            nc.tensor.matmul(
                pt[:],
                self.kT[ds(64 * h, 64), hp, ts(m, P)],
                self.qT[ds(64 * h, 64), hp, ds(qb * SQ, SQ)],
                start=True, stop=True,
            )
            pts.append(pt)
        for h in range(2):
            pt = pts[h]
            if mtile is not None:
                nc.vector.tensor_tensor(
                    pt[:], pt[:], mtile[:, m, :], mybir.AluOpType.add
                )
            if h == 0:
                nc.vector.tensor_scalar_max(attn_t[:, m, h, :], pt[:], 0.0)
            else:
                nc.scalar.activation(
                    attn_t[:, m, h, :], pt[:], mybir.ActivationFunctionType.Relu)

    def av(self, qb, hp, m, attn_t, po):
        nc = self.nc
        for h in range(2):
            nc.tensor.matmul(
                po[ds(64 * h, 64), :],
                self.vN[:, m, ds(128 * hp + 64 * h, 64)],
                attn_t[:, m, h, :],
                start=(m == 0), stop=(m == NM - 1),
            )

    def fc_group(self, sc, eb, tail=False):
        """fc for one (s-chunk, column-half): 2 matmuls -> 1-bank psum ->
        cast into the shared [P,2,SQ] stage; eb==1 fires one 256KB store"""
        nc = self.nc
        pool = self.ps_sc if (tail and (sc + eb) % 2 == 0) else self.ps_fc
        pt = pool.tile([P, SQ], F32, tag="sc" if pool is self.ps_sc else "fc",
                       name=f"fc_{sc}_{eb}")
        for c in range(DLC):
            nc.tensor.matmul(
                pt[:], self.outT[:, c, ts(sc, P)],
                self.wfc_sb[:, c, ds(eb * SQ, SQ)],
                start=(c == 0), stop=(c == DLC - 1),
            )
        if eb == 0:
            self.yt_cur = self.ystage.tile([P, 2, SQ], BF16, tag="yt", name=f"yt_{sc}")
        yt = self.yt_cur
        if tail:
            # engines are free at the tail; split copies and store each half
            # immediately so the ring drains during the remaining matmuls
            nc.vector.tensor_copy(yt[:, eb, 0:SQ // 2], pt[:, 0:SQ // 2])
            nc.scalar.copy(yt[:, eb, SQ // 2:SQ], pt[:, SQ // 2:SQ])
            nc.sync.dma_start(self.y[ts(sc, P), ds(eb * SQ, SQ)], yt[:, eb, :])
        else:
            self.copyback(yt[:, eb, :], pt[:])
            if eb == 1:
                nc.sync.dma_start(
                    self.y[ts(sc, P), :].rearrange("p (e q) -> p e q", e=2),
                    yt[:, :, :])

    def inject(self):
        if self.pe_pending:
            self.pe_pending.pop(0)()

    def load_mask(self, qb):
        if self.maskT is None:
            return None
        nc = self.nc
        mtile = self.mstg.tile([P, NM, SQ], F32, tag="mask", name=f"mask_{qb}")
        for m in range(NM):
            nc.sync.dma_start(
                mtile[:, m, :],
                self.maskT[:, :].rearrange("(m p) q -> p m q", p=P)[:, m, ds(qb * SQ, SQ)],
            )
        return mtile

    # -- main ---------------------------------------------------------------
    def run(self):
        from contextlib import ExitStack

        tc, nc = self.tc, self.nc
        stack = ExitStack()
        sb = stack.enter_context(tc.tile_pool(name="sb", bufs=1))
        # PSUM budget (8 banks): sc 4 (score double-buffer), fc 2
        # (projections + injected work), av 2 (po double buffer)
        self.ps_sc = stack.enter_context(tc.tile_pool(name="ps_sc", bufs=4, space="PSUM"))
        self.ps_fc = stack.enter_context(tc.tile_pool(name="ps_fc", bufs=2, space="PSUM"))
        self.ps_av = stack.enter_context(tc.tile_pool(name="ps_av", bufs=2, space="PSUM"))
        self.attn_pool = stack.enter_context(tc.tile_pool(name="attn", bufs=2))
        self.mstg = stack.enter_context(tc.tile_pool(name="mstg", bufs=2))
        self.ystage = stack.enter_context(tc.tile_pool(name="ystage", bufs=3))

        self.xb = sb.tile([P, NQB, KD, SQ], BF16, name="xb")
        self.wq_sb = sb.tile([P, DLC, KD, P], BF16, name="wq_sb")
        self.wk_sb = sb.tile([P, DLC, KD, P], BF16, name="wk_sb")
        self.wv_sb = sb.tile([P, KD, DL], BF16, name="wv_sb")
        self.wfc_sb = sb.tile([P, DLC, D], BF16, name="wfc_sb")
        self.qT = sb.tile([P, DLC, S], BF16, name="qT")
        self.kT = sb.tile([P, DLC, S], BF16, name="kT")
        self.vN = sb.tile([P, NM, DL], BF16, name="vN")
        self.outT = sb.tile([P, DLC, S], BF16, name="outT")
        self.pe_pending = []

        # loads: ONE fifo queue in exact consumption order (ring arbitration
        # would otherwise let later transfers steal bandwidth from the x
        # stream the head is waiting on). The head emission below matches
        # this order so the PE's in-order stream never waits on a transfer
        # queued behind data it already consumed.
        def ld(dst, src):
            nc.sync.dma_start(dst, src)

        ld(self.wk_sb[:, 0, :, :], self.wk[:, 0, :, :])
        ld(self.xb[:, 0, ds(0, 4), :], self.xT[:, 0, ds(0, 4), :])
        ld(self.wk_sb[:, 1, :, :], self.wk[:, 1, :, :])
        ld(self.xb[:, 0, ds(4, 4), :], self.xT[:, 0, ds(4, 4), :])
        for kh in range(2):
            ld(self.xb[:, 1, ds(kh * 4, 4), :], self.xT[:, 1, ds(kh * 4, 4), :])
        ld(self.wq_sb[:], self.wq[:, :, :, :])
        for kh in range(2):
            ld(self.xb[:, 2, ds(kh * 4, 4), :], self.xT[:, 2, ds(kh * 4, 4), :])
        ld(self.wv_sb[:], self.wv[:, :, :])
        for kh in range(2):
            ld(self.xb[:, 3, ds(kh * 4, 4), :], self.xT[:, 3, ds(kh * 4, 4), :])
        ld(self.wfc_sb[:], self.wfc[:, :, :])

        # PE p-state warmup: dummy matmuls on a memset scratch tile run
        # during the otherwise-idle DMA wait so the ramp to 2.4GHz happens
        # before the first real projection
        scratch = sb.tile([P, SQ], BF16, name="scratch")
        nc.vector.memset(scratch[:], 0.0)
        wp = self.ps_av.tile([P, SQ], F32, tag="av", name="warmup")
        for _ in range(12):
            nc.tensor.matmul(wp[:], scratch[:, 0:P], scratch[:], start=True, stop=True)

        # serial head in arrival order: kT nb0-2, q-projection for qb0,
        # v chunks 0-11, kT nb3, v chunks 12-15
        for nb in range(3):
            for c in range(DLC):
                self.kq_group(self.wk_sb, self.kT, c, nb)
        for c in range(DLC):
            self.kq_group(self.wq_sb, self.qT, c, 0)
        for sc in range(12):
            self.v_group(sc)
        for c in range(DLC):
            self.kq_group(self.wk_sb, self.kT, c, 3)
        for sc in range(12, NM):
            self.v_group(sc)

        # attention loops are software-pipelined ACROSS (qb, hp): each loop's
        # last two av steps and its po copyback slide into the next loop's
        # first two iterations, so the PE stream never drains at boundaries
        def po_copyback(pqb, php, ppo):
            nc.vector.tensor_copy(
                self.outT[:, php, ds(pqb * SQ, SQ // 2)], ppo[:, 0:SQ // 2])
            nc.scalar.copy(
                self.outT[:, php, ds(pqb * SQ + SQ // 2, SQ // 2)],
                ppo[:, SQ // 2:SQ])

        prev = None
        for qb in range(NQB):
            mt = self.load_mask(qb)
            for hp in range(DLC):
                at = self.attn_pool.tile(
                    [P, NM, 2, SQ], BF16, tag="attn", name=f"attn_{qb}_{hp}")
                po = self.ps_av.tile([P, SQ], F32, tag="av", name=f"av_{qb}_{hp}")
                for m in range(NM):
                    self.scores_pair(qb, hp, m, at, mt)
                    if m >= 2:
                        self.av(qb, hp, m - 2, at, po)
                    elif prev is not None:
                        pqb, php, pat, ppo = prev
                        self.av(pqb, php, NM - 2 + m, pat, ppo)
                        if m == 1:
                            po_copyback(pqb, php, ppo)
                    if m % 2 == 1 and m >= 3:
                        self.inject()
                prev = (qb, hp, at, po)
                if hp == 0 and qb < NQB - 1:
                    qn = qb + 1
                    self.pe_pending[:0] = [
                        (lambda c=c, qn=qn: self.kq_group(self.wq_sb, self.qT, c, qn))
                        for c in range(DLC)
                    ]
            if qb < NQB - 1:
                self.pe_pending += [
                    (lambda sc=sc, eb=eb: self.fc_group(sc, eb, tail=False))
                    for sc in range(qb * 4, qb * 4 + 4) for eb in range(2)
                ]
        # drain the final loop's av tail, then its fc
        qb3, hp3, at3, po3 = prev
        for m in (NM, NM + 1):
            self.av(qb3, hp3, m - 2, at3, po3)
        po_copyback(qb3, hp3, po3)
        self.pe_pending += [
            (lambda sc=sc, eb=eb: self.fc_group(sc, eb, tail=True))
            for sc in range(12, 16) for eb in range(2)
        ]
        while self.pe_pending:
            self.inject()

        stack.close()


# ---- host wrapper ---------------------------------------------------------

N_HEAD = 16
_nc_cache = {}


def get_nc(with_mask: bool):
    if with_mask not in _nc_cache:
        _nc_cache[with_mask] = build_nc(with_mask)
    return _nc_cache[with_mask]


def make_in_maps(x, mask, Wq, Wk, Wv, Wfc, with_mask):
    scale = np.float32(1.0 / np.sqrt(D // N_HEAD))
    bf = ml_dtypes.bfloat16
    in_maps = []
    for c in range(8):
        b, hg = divmod(c, 4)
        gs = slice(DL * hg, DL * hg + DL)
        def prearrange(wT, cdim):  # [cdim*128, F] -> [128, cdim, F]
            F = wT.shape[1]
            return np.ascontiguousarray(
                wT.reshape(cdim, P, F).transpose(1, 0, 2)
            ).astype(bf)

        def prearrange_c(wT):  # [KD*128, DLC*128] -> [128, DLC, KD, 128]
            return np.ascontiguousarray(
                wT.reshape(KD, P, DLC, P).transpose(1, 2, 0, 3)
            ).astype(bf)

        xt = x[b].T.reshape(KD, P, NQB, SQ).transpose(1, 2, 0, 3)
        m = {
            "xT": np.ascontiguousarray(xt).astype(bf),
            "wq": prearrange_c((Wq[gs, :] * scale).T),
            "wk": prearrange_c(Wk[gs, :].T),
            "wv": prearrange(Wv[gs, :].T, KD),
            "wfc": prearrange(Wfc[:, gs].T, DLC),
        }
        if with_mask:
            m["maskT"] = np.ascontiguousarray(
                np.broadcast_to(mask, (1, 1, S, S))[0, 0].T.astype(np.float32)
            )
        in_maps.append(m)
    return in_maps


def kernel(x, mask, Wq, Wk, Wv, Wfc, bfc):
    """Full-input entry: shards across 8 trn2 cores, returns the full output."""
    from concourse.bass_utils import run_bass_kernel_spmd

    x = np.asarray(x, dtype=np.float32)
    mask = np.asarray(mask, dtype=np.float32)
    Wq = np.asarray(Wq, dtype=np.float32)
    Wk = np.asarray(Wk, dtype=np.float32)
    Wv = np.asarray(Wv, dtype=np.float32)
    Wfc = np.asarray(Wfc, dtype=np.float32)
    bfc = np.asarray(bfc, dtype=np.float32)

    B = x.shape[0]
    with_mask = bool(np.any(mask))
    nc = get_nc(with_mask)
    in_maps = make_in_maps(x, mask, Wq, Wk, Wv, Wfc, with_mask)

    res = run_bass_kernel_spmd(nc, in_maps, core_ids=list(range(8)))
    parts = np.stack([np.asarray(r["y"], dtype=np.float64) for r in res.results])
    out = parts.reshape(B, 4, S, D).sum(axis=1)
    out += bfc.astype(np.float64)
    return out.astype(np.float32)



# revision 1
# speedup vs baseline: 1.0671x; 1.0671x over previous
"""Bass/Tile kernel for nn_MultiHeadAttention (B=2, S=2048, D=1024, H=16) on 8 trn2 cores.

Sharding: core c -> (b = c//4, head-group hg = c%4). Each core computes 4 heads'
q/k/v projections, relu-attention, and a partial FC (256 of 1024 contraction rows).
Host pre-casts to bf16, pre-arranges x / weight slices, and sums the 4
partials per batch + bias.

v6 design notes:
  - scores: the two heads of a head-pair run CONCURRENTLY in disjoint 64-row
    PE quadrants (tile_position from base partitions), writing one 2-bank
    PSUM tile; ONE 1024-wide relu drains the pair. Vector gets 7 and Scalar 9
    of the 16 relus per m-loop (scalar is faster per element).
  - av: two heads run concurrently in disjoint 64-col output quadrants.
  - attention m-loops are relu/PE balanced (~9.4us each); all remaining PE
    work (v-projections, fc, next q-projection) is injected into the loops.
  - fc computed as (sc)-pairs: 4 matmuls -> 2-bank PSUM -> one 1024-wide
    gpsimd cast-copy -> one 256KB bf16 store with 2KB lines.
  - x arrives as 4 contiguous 1MB nb-block DMAs (8KB/partition lines) on the
    sync hardware queue behind wk; wq+wv ride the scalar hardware queue.
  - y is bf16; host sums partials in fp64.
"""
import numpy as np
import ml_dtypes

import concourse.bass as bass
import concourse.mybir as mybir
import concourse.tile as tile

F32 = mybir.dt.float32
BF16 = mybir.dt.bfloat16
ts, ds = bass.ts, bass.ds

S = 2048
D = 1024
DL = 256      # per-core q/k/v dim (4 heads x 64)
P = 128
KD = D // P   # 8 k-chunks for projections
SQ = 512      # q-block (matmul N)
NQB = S // SQ # 4
NM = S // P   # 16 kpos chunks
DLC = DL // P # 2


def split_excess_waits(nc, max_embed: int = 1):
    """walrus core_v3 codegen accepts at most one sync-wait per instruction;
    move extra waits onto standalone event-sem instructions inserted before."""
    n_split = 0
    counter = 0
    for f in nc.m.functions:
        for blk in f.blocks:
            insts = blk.instructions
            if not any(
                ins.sync_info is not None and len(ins.sync_info.on_wait) > max_embed
                for ins in insts
            ):
                continue
            newl = []
            for ins in insts:
                si = ins.sync_info
                if si is not None and len(si.on_wait) > max_embed:
                    waits = list(si.on_wait)
                    extra, keep = waits[:-max_embed], waits[-max_embed:]
                    for w in extra:
                        counter += 1
                        es = mybir.InstEventSemaphore(name=f"waitsplit_{counter}")
                        es.engine = ins.engine
                        es.sync_info = mybir.SyncInfo(on_wait=[w], on_update=[])
                        newl.append(es)
                        n_split += 1
                    si.on_wait = keep
                newl.append(ins)
            blk.instructions = newl
    return n_split


def build_nc(with_mask: bool):
    nc = bass.Bass()
    # pre-arranged on host: x[p, nb, k, sq] = x.T[128k+p, 512nb+sq] (one
    # contiguous 1MB block per q-block); w[p, c, f] = w.T[128c+p, f]
    xT = nc.dram_tensor("xT", [P, NQB, KD, SQ], BF16, kind="ExternalInput")
    wq = nc.dram_tensor("wq", [P, DLC, KD, P], BF16, kind="ExternalInput")
    wk = nc.dram_tensor("wk", [P, DLC, KD, P], BF16, kind="ExternalInput")
    wv = nc.dram_tensor("wv", [P, KD, DL], BF16, kind="ExternalInput")
    wfc = nc.dram_tensor("wfc", [P, DLC, D], BF16, kind="ExternalInput")
    maskT = nc.dram_tensor("maskT", [S, S], F32, kind="ExternalInput") if with_mask else None
    # bf16 output halves the store traffic; partials are summed in fp64 on host
    y = nc.dram_tensor("y", [S, D], BF16, kind="ExternalOutput")

    with tile.TileContext(nc) as tc:
        _Emitter(tc, xT, wq, wk, wv, wfc, maskT, y).run()
    split_excess_waits(nc)
    return nc


class _Emitter:
    def __init__(self, tc, xT, wq, wk, wv, wfc, maskT, y):
        self.tc = tc
        self.nc = tc.nc
        self.xT, self.wq, self.wk, self.wv, self.wfc = xT, wq, wk, wv, wfc
        self.maskT, self.y = maskT, y
        self.cp = 0

    # -- engine helpers -----------------------------------------------------
    def copyback(self, out_ap, in_ap):
        if self.cp % 2 == 0:
            self.nc.vector.tensor_copy(out_ap, in_ap)
        else:
            self.nc.scalar.copy(out_ap, in_ap)
        self.cp += 1

    # -- emission pieces ----------------------------------------------------
    def kq_group(self, wsb, dstT, c, nb):
        """one projection psum group: dstT[:, c, nb*SQ:...] via 8 k-chunk matmuls"""
        nc = self.nc
        pt = self.ps_fc.tile([P, SQ], F32, tag="fc", name=f"pj_{dstT.name}_{c}_{nb}")
        for k in range(KD):
            nc.tensor.matmul(
                pt[:], wsb[:, c, k, :], self.xb[:, nb, k, :],
                start=(k == 0), stop=(k == KD - 1),
            )
        self.copyback(dstT[:, c, ds(nb * SQ, SQ)], pt[:])

    def v_group(self, sc):
        nc = self.nc
        pt = self.ps_fc.tile([P, DL], F32, tag="fc", name=f"v_{sc}")
        for k in range(KD):
            nc.tensor.matmul(
                pt[:], self.xb[:, sc // 4, k, ds((sc % 4) * P, P)], self.wv_sb[:, k, :],
                start=(k == 0), stop=(k == KD - 1),
            )
        self.copyback(self.vN[:, sc, :], pt[:])

    def scores_pair(self, qb, hp, m, attn_t, mtile):
        """both heads' score matmuls run concurrently in disjoint PE
        row-quadrants; separate 1-bank tiles + 512-wide relus keep the
        drain latency low enough for the 4-slot psum rotation"""
        nc = self.nc
        pts = []
        for h in range(2):
            pt = self.ps_sc.tile([P, SQ], F32, tag="sc", name=f"sc_{qb}_{hp}_{m}_{h}")
            nc.tensor.matmul(
                pt[:],
                self.kT[ds(64 * h, 64), hp, ts(m, P)],
                self.qT[ds(64 * h, 64), hp, ds(qb * SQ, SQ)],
                start=True, stop=True,
            )
            pts.append(pt)
        for h in range(2):
            pt = pts[h]
            if mtile is not None:
                nc.vector.tensor_tensor(
                    pt[:], pt[:], mtile[:, m, :], mybir.AluOpType.add
                )
            if h == 0:
                nc.vector.tensor_scalar_max(attn_t[:, m, h, :], pt[:], 0.0)
            else:
                nc.scalar.activation(
                    attn_t[:, m, h, :], pt[:], mybir.ActivationFunctionType.Relu)

    def av(self, qb, hp, m, attn_t, po):
        nc = self.nc
        for h in range(2):
            nc.tensor.matmul(
                po[ds(64 * h, 64), :],
                self.vN[:, m, ds(128 * hp + 64 * h, 64)],
                attn_t[:, m, h, :],
                start=(m == 0), stop=(m == NM - 1),
            )

    def fc_group(self, sc, eb, tail=False):
        """fc for one (s-chunk, column-half): 2 matmuls -> 1-bank psum ->
        cast into the shared [P,2,SQ] stage; eb==1 fires one 256KB store"""
        nc = self.nc
        pool = self.ps_sc if (tail and (sc + eb) % 2 == 0) else self.ps_fc
        pt = pool.tile([P, SQ], F32, tag="sc" if pool is self.ps_sc else "fc",
                       name=f"fc_{sc}_{eb}")
        for c in range(DLC):
            nc.tensor.matmul(
                pt[:], self.outT[:, c, ts(sc, P)],
                self.wfc_sb[:, c, ds(eb * SQ, SQ)],
                start=(c == 0), stop=(c == DLC - 1),
            )
        if eb == 0:
            self.yt_cur = self.ystage.tile([P, 2, SQ], BF16, tag="yt", name=f"yt_{sc}")
        yt = self.yt_cur
        if tail:
            # engines are free at the tail; split copies and store each half
            # immediately so the ring drains during the remaining matmuls
            nc.vector.tensor_copy(yt[:, eb, 0:SQ // 2], pt[:, 0:SQ // 2])
            nc.scalar.copy(yt[:, eb, SQ // 2:SQ], pt[:, SQ // 2:SQ])
            nc.sync.dma_start(self.y[ts(sc, P), ds(eb * SQ, SQ)], yt[:, eb, :])
        else:
            self.copyback(yt[:, eb, :], pt[:])
            if eb == 1:
                nc.sync.dma_start(
                    self.y[ts(sc, P), :].rearrange("p (e q) -> p e q", e=2),
                    yt[:, :, :])

    def inject(self):
        if self.pe_pending:
            self.pe_pending.pop(0)()

    def load_mask(self, qb):
        if self.maskT is None:
            return None
        nc = self.nc
        mtile = self.mstg.tile([P, NM, SQ], F32, tag="mask", name=f"mask_{qb}")
        for m in range(NM):
            nc.sync.dma_start(
                mtile[:, m, :],
                self.maskT[:, :].rearrange("(m p) q -> p m q", p=P)[:, m, ds(qb * SQ, SQ)],
            )
        return mtile

    # -- main ---------------------------------------------------------------
    def run(self):
        from contextlib import ExitStack

        tc, nc = self.tc, self.nc
        stack = ExitStack()
        sb = stack.enter_context(tc.tile_pool(name="sb", bufs=1))
        # PSUM budget (8 banks): sc 4 (score double-buffer), fc 2
        # (projections + injected work), av 2 (po double buffer)
        self.ps_sc = stack.enter_context(tc.tile_pool(name="ps_sc", bufs=4, space="PSUM"))
        self.ps_fc = stack.enter_context(tc.tile_pool(name="ps_fc", bufs=2, space="PSUM"))
        self.ps_av = stack.enter_context(tc.tile_pool(name="ps_av", bufs=2, space="PSUM"))
        self.attn_pool = stack.enter_context(tc.tile_pool(name="attn", bufs=2))
        self.mstg = stack.enter_context(tc.tile_pool(name="mstg", bufs=2))
        self.ystage = stack.enter_context(tc.tile_pool(name="ystage", bufs=3))

        self.xb = sb.tile([P, NQB, KD, SQ], BF16, name="xb")
        self.wq_sb = sb.tile([P, DLC, KD, P], BF16, name="wq_sb")
        self.wk_sb = sb.tile([P, DLC, KD, P], BF16, name="wk_sb")
        self.wv_sb = sb.tile([P, KD, DL], BF16, name="wv_sb")
        self.wfc_sb = sb.tile([P, DLC, D], BF16, name="wfc_sb")
        self.qT = sb.tile([P, DLC, S], BF16, name="qT")
        self.kT = sb.tile([P, DLC, S], BF16, name="kT")
        self.vN = sb.tile([P, NM, DL], BF16, name="vN")
        self.outT = sb.tile([P, DLC, S], BF16, name="outT")
        self.pe_pending = []

        # loads: ONE fifo queue in exact consumption order (ring arbitration
        # would otherwise let later transfers steal bandwidth from the x
        # stream the head is waiting on). The head emission below matches
        # this order so the PE's in-order stream never waits on a transfer
        # queued behind data it already consumed.
        def ld(dst, src):
            nc.sync.dma_start(dst, src)

        ld(self.wk_sb[:, 0, :, :], self.wk[:, 0, :, :])
        ld(self.xb[:, 0, ds(0, 4), :], self.xT[:, 0, ds(0, 4), :])
        ld(self.wk_sb[:, 1, :, :], self.wk[:, 1, :, :])
        ld(self.xb[:, 0, ds(4, 4), :], self.xT[:, 0, ds(4, 4), :])
        for kh in range(2):
            ld(self.xb[:, 1, ds(kh * 4, 4), :], self.xT[:, 1, ds(kh * 4, 4), :])
        ld(self.wq_sb[:], self.wq[:, :, :, :])
        for kh in range(2):
            ld(self.xb[:, 2, ds(kh * 4, 4), :], self.xT[:, 2, ds(kh * 4, 4), :])
        ld(self.wv_sb[:], self.wv[:, :, :])
        for kh in range(2):
            ld(self.xb[:, 3, ds(kh * 4, 4), :], self.xT[:, 3, ds(kh * 4, 4), :])
        ld(self.wfc_sb[:], self.wfc[:, :, :])

        # PE p-state warmup: dummy matmuls on a memset scratch tile run
        # during the otherwise-idle DMA wait so the ramp to 2.4GHz happens
        # before the first real projection
        scratch = sb.tile([P, SQ], BF16, name="scratch")
        nc.vector.memset(scratch[:], 0.0)
        wp = self.ps_av.tile([P, SQ], F32, tag="av", name="warmup")
        for _ in range(12):
            nc.tensor.matmul(wp[:], scratch[:, 0:P], scratch[:], start=True, stop=True)

        # serial head in arrival order: kT nb0-2, q-projection for qb0,
        # v chunks 0-11, kT nb3, v chunks 12-15
        for nb in range(3):
            for c in range(DLC):
                self.kq_group(self.wk_sb, self.kT, c, nb)
        for c in range(DLC):
            self.kq_group(self.wq_sb, self.qT, c, 0)
        for sc in range(12):
            self.v_group(sc)
        for c in range(DLC):
            self.kq_group(self.wk_sb, self.kT, c, 3)
        for sc in range(12, NM):
            self.v_group(sc)

        # attention loops are software-pipelined ACROSS (qb, hp): each loop's
        # last two av steps and its po copyback slide into the next loop's
        # first two iterations, so the PE stream never drains at boundaries
        def po_copyback(pqb, php, ppo):
            nc.vector.tensor_copy(
                self.outT[:, php, ds(pqb * SQ, SQ // 2)], ppo[:, 0:SQ // 2])
            nc.scalar.copy(
                self.outT[:, php, ds(pqb * SQ + SQ // 2, SQ // 2)],
                ppo[:, SQ // 2:SQ])

        prev = None
        for qb in range(NQB):
            mt = self.load_mask(qb)
            for hp in range(DLC):
                at = self.attn_pool.tile(
                    [P, NM, 2, SQ], BF16, tag="attn", name=f"attn_{qb}_{hp}")
                po = self.ps_av.tile([P, SQ], F32, tag="av", name=f"av_{qb}_{hp}")
                for m in range(NM):
                    self.scores_pair(qb, hp, m, at, mt)
                    if m >= 2:
                        self.av(qb, hp, m - 2, at, po)
                    elif prev is not None:
                        pqb, php, pat, ppo = prev
                        self.av(pqb, php, NM - 2 + m, pat, ppo)
                        if m == 1:
                            po_copyback(pqb, php, ppo)
                    if m % 2 == 1 and m >= 3:
                        self.inject()
                prev = (qb, hp, at, po)
                if hp == 0 and qb < NQB - 1:
                    qn = qb + 1
                    self.pe_pending[:0] = [
                        (lambda c=c, qn=qn: self.kq_group(self.wq_sb, self.qT, c, qn))
                        for c in range(DLC)
                    ]
            if qb < NQB - 1:
                self.pe_pending += [
                    (lambda sc=sc, eb=eb: self.fc_group(sc, eb, tail=False))
                    for sc in range(qb * 4, qb * 4 + 4) for eb in range(2)
                ]
        # drain the final loop's av tail, then its fc
        qb3, hp3, at3, po3 = prev
        for m in (NM, NM + 1):
            self.av(qb3, hp3, m - 2, at3, po3)
        po_copyback(qb3, hp3, po3)
        self.pe_pending += [
            (lambda sc=sc, eb=eb: self.fc_group(sc, eb, tail=True))
            for sc in range(12, 16) for eb in range(2)
        ]
        while self.pe_pending:
            self.inject()

        stack.close()


# ---- host wrapper ---------------------------------------------------------

N_HEAD = 16
_nc_cache = {}


def get_nc(with_mask: bool):
    if with_mask not in _nc_cache:
        _nc_cache[with_mask] = build_nc(with_mask)
    return _nc_cache[with_mask]


def make_in_maps(x, mask, Wq, Wk, Wv, Wfc, with_mask):
    scale = np.float32(1.0 / np.sqrt(D // N_HEAD))
    bf = ml_dtypes.bfloat16
    in_maps = []
    for c in range(8):
        b, hg = divmod(c, 4)
        gs = slice(DL * hg, DL * hg + DL)
        def prearrange(wT, cdim):  # [cdim*128, F] -> [128, cdim, F]
            F = wT.shape[1]
            return np.ascontiguousarray(
                wT.reshape(cdim, P, F).transpose(1, 0, 2)
            ).astype(bf)

        def prearrange_c(wT):  # [KD*128, DLC*128] -> [128, DLC, KD, 128]
            return np.ascontiguousarray(
                wT.reshape(KD, P, DLC, P).transpose(1, 2, 0, 3)
            ).astype(bf)

        xt = x[b].T.reshape(KD, P, NQB, SQ).transpose(1, 2, 0, 3)
        m = {
            "xT": np.ascontiguousarray(xt).astype(bf),
            "wq": prearrange_c((Wq[gs, :] * scale).T),
            "wk": prearrange_c(Wk[gs, :].T),
            "wv": prearrange(Wv[gs, :].T, KD),
            "wfc": prearrange(Wfc[:, gs].T, DLC),
        }
        if with_mask:
            m["maskT"] = np.ascontiguousarray(
                np.broadcast_to(mask, (1, 1, S, S))[0, 0].T.astype(np.float32)
            )
        in_maps.append(m)
    return in_maps


def kernel(x, mask, Wq, Wk, Wv, Wfc, bfc):
    """Full-input entry: shards across 8 trn2 cores, returns the full output."""
    from concourse.bass_utils import run_bass_kernel_spmd

    x = np.asarray(x, dtype=np.float32)
    mask = np.asarray(mask, dtype=np.float32)
    Wq = np.asarray(Wq, dtype=np.float32)
    Wk = np.asarray(Wk, dtype=np.float32)
    Wv = np.asarray(Wv, dtype=np.float32)
    Wfc = np.asarray(Wfc, dtype=np.float32)
    bfc = np.asarray(bfc, dtype=np.float32)

    B = x.shape[0]
    with_mask = bool(np.any(mask))
    nc = get_nc(with_mask)
    in_maps = make_in_maps(x, mask, Wq, Wk, Wv, Wfc, with_mask)

    res = run_bass_kernel_spmd(nc, in_maps, core_ids=list(range(8)))
    parts = np.stack([np.asarray(r["y"], dtype=np.float64) for r in res.results])
    out = parts.reshape(B, 4, S, D).sum(axis=1)
    out += bfc.astype(np.float64)
    return out.astype(np.float32)

